# revision 17
# baseline (speedup 1.0000x reference)
"""Trainium2 Bass kernel for nn_Attention_41729902248209.

8-head attention block: x (8, 512, 32, 32) -> QKV proj -> softmax attention
-> out proj + residual. Data-parallel over batch: one batch element per
NeuronCore (8 cores).

Per-core dataflow (n = 1024 tokens, cin = 512, H = 8 heads, D = 64):
  - everything stays "transposed" (feature dim on partitions) so no on-chip
    transposes are needed anywhere:
      qT, kT : (f' = 64h+d on partitions, n free)   [head pairs share a tile]
      v      : (n on partitions, 64h+d free)
      scoresT: (j on partitions, i free) = k @ qT
      pT     : exp(scoresT) in fp16 (no max subtraction; logits are O(7))
      outT~  : v.T @ pT -> (128, i) in PSUM, TWO heads per matmul via PE
               column tiling (head A -> psum rows 0:64 / array cols 0:64,
               head B -> rows 64:128 / cols 64:128), concurrent streams
      denoms : ones.T @ pT as M=1 matmuls, FOUR streams (2 heads x 2 chunks)
               col-tiled to array col groups 0/32/64/96 -> psum rows
               0/32/64/96 of one bank, all four concurrent
      yT     : W_last.T.T @ outT_scaled + (x + b_last)  (residual, fp32)
  - scores also run pairwise-concurrent: the two heads of a pair occupy PE
    row halves (K=64 at tile_position (0,0) / (64,0)) and their matmuls are
    emitted ADJACENT so the halves stream simultaneously.
  - softmax scale 1/8 is folded into W_q host-side; b_last is folded into the
    residual; b_q/b_k are per-partition DVE adds; b_v is a DVE tensor add.
  - denominators: one batched reciprocal_approx_fast over psum rows 0:97
    (the 4 denom rows live at 0/32/64/96; the bank is memset to 1.0 first so
    the dead rows stay finite), gpsimd partition_broadcast per 64-row half,
    then one (128, 512) DVE multiply per (pair, chunk) straight off PSUM.
  - emission is a software pipeline keyed on ACT density: per head-pair
    "window" the 16 exps stream back-to-back while PE drains a queue of
    background units (next pair's QKV projection, v tiles, previous pair's
    attnv / denominator matmuls) between score steps.
"""

import contextlib
from collections import deque
from functools import partial

import numpy as np

import concourse.mybir as mybir
import concourse.tile as tile
from concourse import bacc
from concourse.bass_utils import run_bass_kernel_spmd

F16 = mybir.dt.float16
F32 = mybir.dt.float32

BS = 8
H = 8
D = 64
CIN = 512
N = 1024
NK = CIN // 128  # contraction tiles for cin
NJT = N // 128  # j tiles
NCH = N // 512  # i chunks of 512

AF = mybir.ActivationFunctionType
ALU = mybir.AluOpType


def _emit(tc, d, sb, ps):
    nc = tc.nc

    x16_sb = sb.tile([128, NK * N], F16, tag="x16")
    xr_sb = sb.tile([128, NK * N], F16, tag="xr")
    wq_sb = sb.tile([128, NK * 512], F16, tag="wq")
    wk_sb = sb.tile([128, NK * 512], F16, tag="wk")
    wv_sb = sb.tile([128, NK * 512], F16, tag="wv")
    wl_sb = sb.tile([128, NK * 512], F16, tag="wl")
    bqk_sb = sb.tile([128, 8], F32, tag="bqk")
    bvb_sb = sb.tile([128, 512], F32, tag="bvb")
    ones_sb = sb.tile([128, 1], F16, tag="ones")
    zrow_sb = sb.tile([128, 512], F16, tag="zrow")
    qT_sb = sb.tile([128, 4 * N], F16, tag="qT")
    kT_sb = sb.tile([128, 4 * N], F16, tag="kT")
    v_sb = sb.tile([128, NJT * 512], F16, tag="v")
    os_sb = sb.tile([128, 4 * N], F16, tag="outT_s")

    # --- input DMAs (ktile k of a (512, W) dram tensor -> cols [W*k, W*k+W)) ---
    # Issue is the bottleneck (one sequencer = ~0.65us per DMA, serial), so
    # spread the loads across idle engines' DGE queues: sync takes wq, gpsimd
    # takes x16, scalar takes wk (ACT idle until the first exp), and the
    # late-needed tensors follow behind on each queue.
    for k in range(NK):
        r = slice(128 * k, 128 * k + 128)
        nc.sync.dma_start(wq_sb[:, 512 * k : 512 * k + 512], d["wq"].ap()[r, :])
        nc.gpsimd.dma_start(x16_sb[:, N * k : N * k + N], d["x16"].ap()[r, :])
        nc.scalar.dma_start(wk_sb[:, 512 * k : 512 * k + 512], d["wk"].ap()[r, :])
    nc.sync.dma_start(bqk_sb[:], d["bqk"].ap())
    for k in range(NK):
        r = slice(128 * k, 128 * k + 128)
        nc.sync.dma_start(wv_sb[:, 512 * k : 512 * k + 512], d["wv"].ap()[r, :])
    nc.gpsimd.dma_start(bvb_sb[:], d["bvb"].ap())
    nc.vector.memset(ones_sb[:], 1.0)
    nc.vector.memset(zrow_sb[:], 0.0)
    for k in range(NK):
        r = slice(128 * k, 128 * k + 128)
        nc.scalar.dma_start(wl_sb[:, 512 * k : 512 * k + 512], d["wl"].ap()[r, :])
        nc.gpsimd.dma_start(xr_sb[:, N * k : N * k + N], d["xr"].ap()[r, :])

    # --- pools that emitters close over ---
    stack = contextlib.ExitStack()
    pt_pool = stack.enter_context(tc.tile_pool(name="pt", bufs=4))
    rr_pool = stack.enter_context(tc.tile_pool(name="rr", bufs=1))
    rb_pool = stack.enter_context(tc.tile_pool(name="rb", bufs=4))
    y_pool = stack.enter_context(tc.tile_pool(name="y", bufs=3))

    pt_tiles = {}
    pv_tiles = {}
    dp_tiles = {}
    rb_tiles = {}

    # --- stage emitters ---
    def qk_quarter(t, dst, wsb, bcol, c):
        """One (128, 512) chunk of the q or k projection for f'-tile t."""
        p = ps.tile([128, 512], F32, tag="mm", bufs=2, name=f"qk{t}_{bcol}_{c}")
        for k in range(NK):
            nc.tensor.matmul(
                p[:],
                wsb[:, 512 * k + 128 * t : 512 * k + 128 * t + 128],
                x16_sb[:, N * k + 512 * c : N * k + 512 * c + 512],
                start=(k == 0),
                stop=(k == NK - 1),
            )
        nc.vector.tensor_scalar_add(
            dst[:, N * t + 512 * c : N * t + 512 * c + 512],
            p[:],
            bqk_sb[:, bcol : bcol + 1],
        )

    def v_tile(jt):
        """Project v for token tile jt: (128 tokens, 512 feats) + b_v."""
        p = ps.tile([128, 512], F32, tag="mm", bufs=2, name=f"vp{jt}")
        for k in range(NK):
            nc.tensor.matmul(
                p[:],
                x16_sb[:, N * k + 128 * jt : N * k + 128 * jt + 128],
                wv_sb[:, 512 * k : 512 * k + 512],
                start=(k == 0),
                stop=(k == NK - 1),
            )
        nc.vector.tensor_tensor(
            v_sb[:, 512 * jt : 512 * jt + 512], p[:], bvb_sb[:], ALU.add
        )

    def scores_step(pr, jt):
        """Scores j-tile jt for head pair (2pr, 2pr+1) + exps.

        The two heads' matmuls are adjacent with row tile_positions (0,0) /
        (64,0) so the PE row halves run concurrently."""
        sps = []
        for hh in (0, 1):
            h = 2 * pr + hh
            if jt == 0:
                pt_tiles[h] = pt_pool.tile(
                    [128, NJT * N], F16, tag="pt", name=f"pt{h}"
                )
            sp = ps.tile([128, N], F32, tag="score", bufs=2, name=f"sp{h}_{jt}")
            sps.append(sp)
        for c in range(NCH):
            for hh in (0, 1):
                po = 64 * hh
                nc.tensor.matmul(
                    sps[hh][:, 512 * c : 512 * c + 512],
                    kT_sb[po : po + 64, N * pr + 128 * jt : N * pr + 128 * jt + 128],
                    qT_sb[po : po + 64, N * pr + 512 * c : N * pr + 512 * c + 512],
                    start=True,
                    stop=True,
                    tile_position=(po, 0),
                )
        for hh in (0, 1):
            h = 2 * pr + hh
            nc.scalar.activation(pt_tiles[h][:, N * jt : N * jt + N], sps[hh][:], AF.Exp)

    def bank_bracket(p, start):
        """Dummy K=1 matmul with zero weights: opens (start=True, N=512 —
        zeroes the whole bank so every stream's bytes are uniformly
        non-pending and accumulate from 0) or closes (stop=True, N=1 — adds
        0 to col 0) the accumulation group of a bank shared by several
        col-tiled streams. The full-partition write also forces the
        scheduler to order it before/after every stream (W-W overlap)."""
        if start:
            nc.tensor.matmul(
                p[:, 0:512],
                zrow_sb[0:1, 0:128],
                zrow_sb[0:1, 0:512],
                start=True,
                stop=False,
            )
        else:
            nc.tensor.matmul(
                p[:, 0:1],
                zrow_sb[0:1, 0:128],
                zrow_sb[0:1, 0:1],
                start=False,
                stop=True,
            )

    def attnv_unit(pr, c, g2):
        """2 j-tiles of outT~ for pair pr, chunk c: both heads concurrently
        via col tiles (0,0) / (0,64); head A -> psum rows 0:64, B -> 64:128."""
        key = (pr, c)
        if key not in pv_tiles:
            pv_tiles[key] = ps.tile(
                [128, 512], F32, tag="av", bufs=2, name=f"av{pr}_{c}"
            )
            bank_bracket(pv_tiles[key], start=True)
        p = pv_tiles[key]
        for jt in (2 * g2, 2 * g2 + 1):
            for hh in (0, 1):
                h = 2 * pr + hh
                nc.tensor.matmul(
                    p[64 * hh : 64 * hh + 64, :],
                    v_sb[:, 512 * jt + 64 * h : 512 * jt + 64 * h + 64],
                    pt_tiles[h][:, N * jt + 512 * c : N * jt + 512 * c + 512],
                    start=False,
                    stop=False,
                    tile_position=(0, 64 * hh),
                )
        if g2 == 3:
            bank_bracket(p, start=False)

    def denom_begin(pr):
        """Allocate + open the denominator bank for pair pr."""
        dp = ps.tile([128, 512], F32, tag="mm", bufs=2, name=f"dp{pr}")
        dp_tiles[pr] = dp
        bank_bracket(dp, start=True)

    def denom_unit(pr, g2):
        """2 j-tiles of all four denominator streams (2 heads x 2 chunks),
        col-tiled to groups 0/32/64/96 -> concurrent M=1 matmuls. The bank's
        accumulation group is bracketed by bank_bracket dummies."""
        dp = dp_tiles[pr]
        for jt in (2 * g2, 2 * g2 + 1):
            for hh in (0, 1):
                for c in range(NCH):
                    g = 2 * hh + c
                    nc.tensor.matmul(
                        dp[32 * g : 32 * g + 1, :],
                        ones_sb[:, 0:1],
                        pt_tiles[2 * pr + hh][
                            :, N * jt + 512 * c : N * jt + 512 * c + 512
                        ],
                        start=False,
                        stop=False,
                        tile_position=(0, 32 * g),
                    )
        if g2 == 3:
            bank_bracket(dp, start=False)

    def extract_a(pr):
        """Denominator rows (psum partitions 0/32/64/96) -> partition-0 SBUF
        copies -> reciprocal_approx_fast (custom DVE ops only work at
        partition 0) -> gpsimd broadcasts (source col offset must stay small,
        so one narrow tile per row) into per-chunk (128, 512) factor tiles."""
        dp = dp_tiles.pop(pr)
        rcs = []
        for g in range(4):
            rr = rr_pool.tile(
                [128, 1024], F32, tag=f"rr{g}", bufs=1, name=f"rr{pr}_{g}"
            )
            nc.vector.tensor_copy(rr[0:1, 0:512], dp[32 * g : 32 * g + 1, :])
            nc.vector.reciprocal_approx_fast(rr[0:1, 512:1024], rr[0:1, 0:512])
            rcs.append(rr)
        for c in range(NCH):
            for hh in (0, 1):
                # dst must start at partition 0 (gpsimd ucode limitation), so
                # each head gets its own full-height factor tile
                rb = rb_pool.tile(
                    [128, 512], F32, tag="rb", name=f"rb{pr}_{c}_{hh}"
                )
                nc.gpsimd.partition_broadcast(rb[:], rcs[2 * hh + c][0:1, 512:1024])
                rb_tiles[(pr, c, hh)] = rb

    def extract_b(pr):
        """Scale outT~ by 1/denom: one (64, 512) multiply per (chunk, head)."""
        for c in range(NCH):
            p = pv_tiles.pop((pr, c))
            for hh in (0, 1):
                rb = rb_tiles.pop((pr, c, hh))
                sl = slice(64 * hh, 64 * hh + 64)
                nc.vector.tensor_tensor(
                    os_sb[sl, N * pr + 512 * c : N * pr + 512 * c + 512],
                    p[sl, :],
                    rb[sl, :],
                    ALU.mult,
                )

    def outproj(ct):
        """yT c-tile ct: W_lastT.T @ outT_s + (x + b_last), fp32 out + DMA."""
        p = ps.tile([128, N], F32, tag="score", bufs=2, name=f"yp{ct}")
        for c in range(NCH):
            for k in range(NK):
                nc.tensor.matmul(
                    p[:, 512 * c : 512 * c + 512],
                    wl_sb[:, 512 * k + 128 * ct : 512 * k + 128 * ct + 128],
                    os_sb[:, N * k + 512 * c : N * k + 512 * c + 512],
                    start=(k == 0),
                    stop=(k == NK - 1),
                )
        if ct < 3:
            y = y_pool.tile([128, N], F16, tag="y")
            nc.vector.tensor_tensor(
                y[:], p[:], xr_sb[:, N * ct : N * ct + N], ALU.add
            )
            nc.sync.dma_start(d["y"].ap()[128 * ct : 128 * ct + 128, :], y[:])
        else:
            # split the last c-tile so its evac/DMA pipeline drains earlier
            for c in range(NCH):
                sl = slice(512 * c, 512 * c + 512)
                y = y_pool.tile([128, 512], F16, tag="y2", name=f"y3_{c}")
                nc.vector.tensor_tensor(
                    y[:],
                    p[:, sl],
                    xr_sb[:, N * ct + 512 * c : N * ct + 512 * c + 512],
                    ALU.add,
                )
                nc.sync.dma_start(d["y"].ap()[128 * ct : 128 * ct + 128, sl], y[:])

    # --- software-pipelined emission -------------------------------------
    def qk_tile_units(t):
        return [
            partial(qk_quarter, t, dst, wsb, bcol, c)
            for (dst, wsb, bcol) in ((qT_sb, wq_sb, t), (kT_sb, wk_sb, 4 + t))
            for c in range(NCH)
        ]

    def pair_drain_units(pr):
        """attnv + denominators + extraction for pair pr, ordered so the
        denominator matmul block is contiguous (dp's mm-ring slot is held
        briefly) and extract_b trails extract_a by two attnv units (the
        gpsimd broadcasts complete while DVE stays busy elsewhere)."""
        units = []
        units += [partial(attnv_unit, pr, 0, g2) for g2 in range(4)]
        units += [partial(attnv_unit, pr, 1, g2) for g2 in (0, 1)]
        units.append(partial(denom_begin, pr))
        units += [partial(denom_unit, pr, g2) for g2 in range(4)]
        units.append(partial(extract_a, pr))
        units += [partial(attnv_unit, pr, 1, g2) for g2 in (2, 3)]
        units.append(partial(extract_b, pr))
        return units

    for u in qk_tile_units(0):
        u()

    queue = deque()
    queue.extend(qk_tile_units(1))
    queue.extend(partial(v_tile, jt) for jt in range(4))

    for pr in range(4):
        if pr >= 1:
            if pr == 1:
                queue.extend(partial(v_tile, jt) for jt in range(4, 8))
            if pr + 1 <= 3:
                queue.extend(qk_tile_units(pr + 1))
            queue.extend(pair_drain_units(pr - 1))
        for jt in range(NJT):
            scores_step(pr, jt)
            for _ in range(3):
                if queue:
                    queue.popleft()()
    while queue:
        queue.popleft()()
    for u in pair_drain_units(3):
        u()
    for ct in range(4):
        outproj(ct)

    stack.close()


def _build(loop=1):
    nc = bacc.Bacc("TRN2", target_bir_lowering=False, debug=False, num_devices=BS)
    d = {}
    d["x16"] = nc.dram_tensor("x16", [CIN, N], F16, kind="ExternalInput")
    d["xr"] = nc.dram_tensor("xr", [CIN, N], F16, kind="ExternalInput")
    d["wq"] = nc.dram_tensor("wq", [CIN, 512], F16, kind="ExternalInput")
    d["wk"] = nc.dram_tensor("wk", [CIN, 512], F16, kind="ExternalInput")
    d["wv"] = nc.dram_tensor("wv", [CIN, 512], F16, kind="ExternalInput")
    d["wl"] = nc.dram_tensor("wl", [CIN, 512], F16, kind="ExternalInput")
    d["bqk"] = nc.dram_tensor("bqk", [128, 8], F32, kind="ExternalInput")
    d["bvb"] = nc.dram_tensor("bvb", [128, 512], F32, kind="ExternalInput")
    d["y"] = nc.dram_tensor("y", [CIN, N], F16, kind="ExternalOutput")

    with tile.TileContext(nc) as tc:
        with (
            tc.tile_pool(name="sb", bufs=1) as sb,
            tc.tile_pool(name="ps", bufs=4, space="PSUM") as ps,
        ):
            for i in range(loop):
                if i:
                    with tc.tile_critical():
                        nc.all_engine_barrier()
                _emit(tc, d, sb, ps)
    nc.compile()
    return nc


_NC_CACHE = {}


def get_nc(loop=1):
    if loop not in _NC_CACHE:
        _NC_CACHE[loop] = _build(loop)
    return _NC_CACHE[loop]


def host_prep(x, W_fc, b_fc, W_last, b_last):
    """Full inputs -> list of 8 per-core input maps."""
    x = np.asarray(x, dtype=np.float32)
    W_fc = np.asarray(W_fc, dtype=np.float32)
    b_fc = np.asarray(b_fc, dtype=np.float32)
    W_last = np.asarray(W_last, dtype=np.float32)
    b_last = np.asarray(b_last, dtype=np.float32)

    hh = np.arange(H).repeat(D) * 3 * D  # 192h per f'=64h+d
    dd = np.tile(np.arange(D), H)
    pq, pk, pv = hh + dd, hh + D + dd, hh + 2 * D + dd

    wq = np.ascontiguousarray((W_fc[pq] * 0.125).T).astype(np.float16)
    wk = np.ascontiguousarray(W_fc[pk].T).astype(np.float16)
    wv = np.ascontiguousarray(W_fc[pv].T).astype(np.float16)
    wl = np.ascontiguousarray(W_last.T).astype(np.float16)
    bq, bk, bv = b_fc[pq] * 0.125, b_fc[pk], b_fc[pv]
    bqk = np.ascontiguousarray(
        np.concatenate([bq.reshape(4, 128).T, bk.reshape(4, 128).T], axis=1)
    ).astype(np.float32)
    bvb = np.ascontiguousarray(np.tile(bv[None, :], (128, 1))).astype(np.float32)

    xf = x.reshape(BS, CIN, N)
    maps = []
    for b in range(BS):
        maps.append(
            {
                "x16": xf[b].astype(np.float16),
                "xr": (xf[b] + b_last[:, None]).astype(np.float16),
                "wq": wq,
                "wk": wk,
                "wv": wv,
                "wl": wl,
                "bqk": bqk,
                "bvb": bvb,
            }
        )
    return maps


def kernel(x, W_fc, b_fc, W_last, b_last):
    nc = get_nc()
    maps = host_prep(x, W_fc, b_fc, W_last, b_last)
    res = run_bass_kernel_spmd(nc, maps, core_ids=list(range(BS)))
    y = np.stack([res.results[b]["y"].astype(np.float32) for b in range(BS)])
    return y.reshape(BS, CIN, 32, 32)
